# revision 39
# baseline (speedup 1.0000x reference)
"""GQA causal self-attention with ALiBi — Trainium2 Bass kernel, 8 NeuronCores.

Sharding: one (batch, kv-head) pair per core (2 batches x 4 kv heads = 8 cores).
Each core computes its 4 query heads' attention over the full sequence and a
partial output projection y_partial = att_heads @ Wo[head_rows]; the host sums
the 4 partials per batch.

Key structural facts exploited:
- ALiBi slopes on these cores are >= 0.25, so keys more than 64 positions back
  are weighted < e^(2.5-16): each query needs only its causal key block and
  the previous one. Key-block-major processing with a 192-query span per key
  block gives every query a window of [64, 191] keys.
- All 4 query heads share one KV head (GQA): S^T for all 4 heads lives in one
  PSUM supertile (one exp, one causal mask per key block), and the PV matmul
  for all 4 heads is a single instruction (shared V^T stationary operand, 3D
  moving AP over heads x queries).

Per key block kt (keys [128kt, 128kt+128), queries [128kt, 128kt+192)):
  S^T[j, h, i] = q_hi . k_j - (s*i + SHIFT)    (aug row; shift cancels in y)
  P = exp(S^T + s*j)  -> pt[kt] bf16           (s*j exact via ACT bias)
  causal mask: one affine_select fill(0) on the diagonal 128 block (fill, not
  multiply: masked entries hold exp-overflow inf and 0*inf would be NaN)
  PV for query block qt: osum bank += V^T[kt-1] @ pt[kt-1][tail 128:256]
                                   + V^T[kt]   @ pt[kt][diag 0:128]
  (pt columns [192:256) are permanently zero, so the tail matmul is a clean
  full-width accumulation; osum layout [65, 256q, 4h] puts each query block
  in exactly one PSUM bank.)
Normalization: l comes from the ones-column of [V^T | 1]; 1/l is broadcast
across partitions via a 0/1 selection matmul (esel @ lrows) and applied with
one tensor_tensor mult per half of the att tile.
"""

import math
import numpy as np
import ml_dtypes

import concourse.bass as bass
import concourse.mybir as mybir
import concourse.tile as tile
from concourse import bacc
from concourse.bass_utils import run_bass_kernel_spmd

f32 = mybir.dt.float32
f32r = mybir.dt.float32r
bf16 = mybir.dt.bfloat16
npbf16 = ml_dtypes.bfloat16
EXP = mybir.ActivationFunctionType.Exp

B, T, C = 2, 2048, 1024
H, HKV, HD = 16, 4, 64
G = H // HKV              # 4 query heads per core
GH = G * HD               # 256
QKV = GH + 2 * HD         # 384 projection cols per core
SCALE = 1.0 / math.sqrt(HD)
SHIFT = 4.0
NKT = T // 128            # 16 key blocks of 128
SPAN = 176                # queries touched per key block (window >= 48)

_CACHED_NC = None


def _build_nc(reps=1):
    nc = bacc.Bacc("TRN2", target_bir_lowering=False, debug=False)

    xT = nc.dram_tensor("xT", [C, T], bf16, kind="ExternalInput")
    wqkv = nc.dram_tensor("wqkv", [C, QKV], bf16, kind="ExternalInput")
    wo = nc.dram_tensor("wo", [GH, C], bf16, kind="ExternalInput")
    aux = nc.dram_tensor("aux", [8, T], bf16, kind="ExternalInput")
    sjcol = nc.dram_tensor("sjcol", [128, NKT], f32, kind="ExternalInput")
    # y partials in bf16: host sums 4 partials per batch in fp32.
    y = nc.dram_tensor("y", [T, C], bf16, kind="ExternalOutput")

    with tile.TileContext(nc) as tc:
        import contextlib
        with contextlib.ExitStack() as bctx:
            cpool = bctx.enter_context(tc.tile_pool(name="hoisted", bufs=1))
            hoisted = _emit_hoisted(nc, cpool)
            hoisted["ypool"] = bctx.enter_context(
                tc.tile_pool(name="ypool", bufs=2))
            # PSUM budget (8 banks): sp 2x2 + osum 1x2 + shared 2x1
            hoisted["sppool"] = bctx.enter_context(
                tc.tile_pool(name="sppool", bufs=2, space="PSUM"))
            hoisted["ospool"] = bctx.enter_context(
                tc.tile_pool(name="ospool", bufs=1, space="PSUM"))
            hoisted["pssm"] = bctx.enter_context(
                tc.tile_pool(name="pssm", bufs=2, space="PSUM"))
            pending = []
            for r in range(reps):
                pending = _emit(nc, tc, xT, wqkv, wo, aux, sjcol, y, hoisted,
                                pending, sfx=f"_{r}" if r else "")
            for fn in pending:
                fn()

    nc.finalize()
    return nc


def _emit_hoisted(nc, const):
    """Constant scratch shared by every rep: built/zeroed exactly once.

    The zeros in pt_all's pad columns, lrows' unused rows, and v_sb's ones
    column are never overwritten by the per-rep body, so later reps reuse
    them without re-initialization.
    """
    KA = 65
    # P tiles: one region per key block; columns [SPAN:256) stay zero so the
    # PV tail matmul can read a full 128-wide slice.
    pt_all = const.tile([128, NKT, G, 256], bf16, name="pt_all")
    nc.gpsimd.memset(pt_all[:, :, :, SPAN:256], 0.0)

    # V^T blocks + ones column (l accumulator)
    v_sb = const.tile([128, NKT, HD + 1], bf16, name="v_sb")
    for kt in range(NKT):
        nc.vector.memset(v_sb[:, kt, HD:HD + 1], 1.0)

    # 1/l broadcast scaffolding: rows {0,32,64,96} of lrows get scattered
    # 1/l values; all other rows must stay zero (esel's zero rows multiply
    # them, and 0*garbage from uninitialized SBUF would poison rp).
    lrows = const.tile([128, T], f32r, name="lrows")
    nc.gpsimd.memset(lrows.bitcast(f32), 0.0)

    ident_f = const.tile([64, 64], f32, name="ident_f")
    nc.gpsimd.memset(ident_f, 0.0)
    nc.gpsimd.affine_select(
        out=ident_f, in_=ident_f, compare_op=mybir.AluOpType.not_equal,
        fill=1.0, base=0, pattern=[[-1, 64]], channel_multiplier=1)
    ident = const.tile([64, 64], bf16, name="ident")
    nc.vector.tensor_copy(ident, ident_f)

    # 0/1 head-selection matrices for the 1/l broadcast matmul
    esel = []
    for c in range(2):
        e = const.tile([128, 128], f32r, name=f"esel{c}")
        nc.vector.memset(e.bitcast(f32), 0.0)
        nc.vector.memset(e[64 * c:64 * c + 1, 0:64].bitcast(f32), 1.0)
        nc.vector.memset(e[64 * c + 32:64 * c + 33, 64:128].bitcast(f32), 1.0)
        esel.append(e)

    att = [const.tile([128, T], bf16, name=f"att{c}") for c in range(2)]
    ls4 = const.tile([KA, G, T], f32r, name="ls4")

    # persistent I/O staging reused by every rep (cross-rep WAR deps let the
    # next rep's loads prefetch during this rep's tail)
    wqkv_sb = const.tile([128, C // 128, QKV], bf16, name="wqkv_sb")
    sj_sb = const.tile([128, NKT], f32, name="sj_sb")
    kaug = const.tile([KA, T], bf16, name="kaug")
    q4 = const.tile([KA, G, T], bf16, name="q4")
    wo_sb = const.tile([128, GH // 128, C], bf16, name="wo_sb")
    xts = {(tc2, nn): const.tile([128, 8, 512], bf16, name=f"xt{tc2}_{nn}")
           for tc2 in range(2) for nn in range(2)}
    vts = {(tc2, nn): const.tile([64, 512], bf16, name=f"vt{tc2}_{nn}")
           for tc2 in range(2) for nn in range(2)}
    return dict(pt_all=pt_all, v_sb=v_sb, lrows=lrows,
                ident=ident, esel=esel, att=att, ls4=ls4,
                wqkv_sb=wqkv_sb, sj_sb=sj_sb, kaug=kaug, q4=q4,
                wo_sb=wo_sb, xts=xts, vts=vts)


def _emit(nc, tc, xT, wqkv, wo, aux, sjcol, y, hoisted, pending, sfx=""):
    import contextlib
    ctx = contextlib.ExitStack()
    pt_all = hoisted["pt_all"]
    v_sb = hoisted["v_sb"]
    lrows = hoisted["lrows"]
    ident = hoisted["ident"]
    esel = hoisted["esel"]
    att = hoisted["att"]
    ls4 = hoisted["ls4"]
    wqkv_sb = hoisted["wqkv_sb"]
    sj_sb = hoisted["sj_sb"]
    kaug = hoisted["kaug"]
    q4 = hoisted["q4"]
    wo_sb = hoisted["wo_sb"]
    _xts = hoisted["xts"]
    _vts = hoisted["vts"]
    ypool = hoisted["ypool"]
    sppool = hoisted["sppool"]
    ospool = hoisted["ospool"]
    pssm = hoisted["pssm"]
    lrows_r = lrows.rearrange("(o p) t -> o p t", p=32)
    _xts_cache = {0: [_xts[(0, 0)], _xts[(0, 1)]],
                  1: [_xts[(1, 0)], _xts[(1, 1)]]}
    with ctx:
        # ---- input loads. All transfers serialize on one DMA stream:
        # order = priority; the first wqkv/x tiles are split in half so
        # compute starts early.
        wqkv_r = wqkv.rearrange("(o p) m -> p o m", p=128)
        xsrc = xT.rearrange("(o p) t -> p o t", p=128)
        nc.sync.dma_start(wqkv_sb[:, 0:4, :], wqkv_r[:, 0:4, :])
        nc.sync.dma_start(_xts[(0, 0)][:, 0:4, :], xsrc[:, 0:4, 0:512])
        nc.sync.dma_start(wqkv_sb[:, 4:8, :], wqkv_r[:, 4:8, :])
        nc.sync.dma_start(_xts[(0, 0)][:, 4:8, :], xsrc[:, 4:8, 0:512])
        nc.gpsimd.dma_start(_xts[(0, 1)], xsrc[:, :, 512:1024])

        nc.scalar.dma_start(sj_sb, sjcol[:, :])

        # 65 = 64 k/q features + one augmentation row: kaug row 64 is all
        # ones, qaug row 64 is -(s*i + SHIFT); their product applies the
        # per-query stabilizing shift inside the S^T matmul (bf16 rounding of
        # the shift cancels exactly between numerator and l). q4 holds all
        # four heads; rows 4:8 of aux are the same negm repeated so one DMA
        # fills all four aug rows.
        nc.scalar.dma_start(kaug[64:65, :], aux[0:1, :])   # ones
        nc.scalar.dma_start(q4[64:65, :, :], aux[4:8, :])  # negm x4

        nc.sync.dma_start(_xts[(1, 0)], xsrc[:, :, 1024:1536])
        nc.sync.dma_start(_xts[(1, 1)], xsrc[:, :, 1536:2048])
        nc.scalar.dma_start(wo_sb, wo.rearrange("(o p) n -> p o n", p=128))

        # ---- phase B: QKV^T projection, one (mt, nn) unit at a time.
        # PSUM->SBUF evacuations split: q rows on ACT, k/v rows on DVE.
        # V transposes are deferred (emit_b_vt) so they don't stall PE
        # behind the ACT copy producing vt. ----
        def emit_b_unit(tc2, mt, nn):
            tcol = tc2 * 1024
            xts = _xts_cache[tc2]
            pcol = tcol + nn * 512
            pb = pssm.tile([128, 512], f32, name=f"pqkv{tc2}_{mt}_{nn}", tag="sm")
            for c8 in range(8):
                nc.tensor.matmul(
                    pb,
                    lhsT=wqkv_sb[:, c8, mt * 128:(mt + 1) * 128],
                    rhs=xts[nn][:, c8, :],
                    start=(c8 == 0), stop=(c8 == 7))
            if mt < 2:
                nc.scalar.copy(q4[0:64, 2 * mt, pcol:pcol + 512], pb[0:64, :])
                nc.scalar.copy(q4[0:64, 2 * mt + 1, pcol:pcol + 512], pb[64:128, :])
            else:
                nc.vector.tensor_copy(kaug[0:64, pcol:pcol + 512], pb[0:64, :])
                nc.scalar.copy(_vts[(tc2, nn)], pb[64:128, :])

        def emit_b_vt(tc2, nn):
            vt = _vts[(tc2, nn)]
            for i in range(4):
                pt_ps = pssm.tile([128, 64], bf16,
                                  name=f"ptr{tc2}_{nn}_{i}", tag="sm")
                nc.tensor.transpose(pt_ps, vt[:, i * 128:(i + 1) * 128], ident)
                nc.vector.tensor_copy(
                    v_sb[:, tc2 * 8 + nn * 4 + i, 0:HD], pt_ps)

        # ---- phase C pieces ----
        def emit_s(kt):
            span = min(SPAN, T - 128 * kt)
            sp = sppool.tile([128, G, 256], f32, name=f"sp{kt}", tag="sp")
            for h in range(G):
                nc.tensor.matmul(
                    sp[:, h, 0:span],
                    lhsT=kaug[:, kt * 128:(kt + 1) * 128],
                    rhs=q4[:, h, 128 * kt:128 * kt + span],
                    start=True, stop=True)
            pt = pt_all[:, kt, :, :]
            nc.scalar.activation(pt[:, :, 0:span], sp[:, :, 0:span], EXP,
                                 bias=sj_sb[:, kt:kt + 1])
            # zero P where query < key inside the diagonal block. Must be a
            # fill (not a mask multiply): masked entries hold exp-overflow
            # inf, and 0*inf would make NaN.
            nc.gpsimd.affine_select(
                out=pt[:, :, 0:128], in_=pt[:, :, 0:128],
                compare_op=mybir.AluOpType.is_ge, fill=0.0,
                base=0, pattern=[[0, G], [1, 128]], channel_multiplier=-1)

        _osum = [None]

        def emit_pv(qt):
            # each query block qt occupies bank qt%2 of the half-chunk osum
            if qt % 2 == 0:
                _osum[0] = ospool.tile([HD + 1, 2, 128, G], f32,
                                       name=f"osum{qt}", tag="osum")
            osum = _osum[0]
            ob = osum[:, qt % 2, :, :]
            if qt > 0:
                nc.tensor.matmul(
                    ob, lhsT=v_sb[:, qt - 1, :],
                    rhs=pt_all[:, qt - 1, :, 128:256].rearrange("p h q -> p q h"),
                    start=True, stop=False)
            nc.tensor.matmul(
                ob, lhsT=v_sb[:, qt, :],
                rhs=pt_all[:, qt, :, 0:128].rearrange("p h q -> p q h"),
                start=(qt == 0), stop=True)

        def emit_recip(qt):
            # per-query-block reciprocal of l straight off the fresh osum bank
            osum = _osum[0]
            q0 = 128 * qt
            with nc.allow_low_precision(reason="softmax reciprocal to fp32r"):
                nc.vector.reciprocal(
                    ls4[64:65, :, q0:q0 + 128].rearrange("p h q -> p q h"),
                    osum[HD:HD + 1, qt % 2, :, :])

        def emit_evac(qt1):
            # evacuate the half-chunk [128*(qt1-1), 128*qt1+128) after PV(qt1)
            osum = _osum[0]
            q0 = 128 * (qt1 - 1)
            nc.sync.dma_start(
                lrows_r[:, 0:1, q0:q0 + 256],
                ls4[64:65, :, q0:q0 + 256])
            for h in range(G):
                c2, half = h // 2, (h % 2) * 64
                if h < 2:
                    nc.scalar.copy(att[c2][half:half + 64, q0:q0 + 256],
                                   osum[0:HD, :, :, h])
                else:
                    nc.vector.tensor_copy(att[c2][half:half + 64, q0:q0 + 256],
                                          osum[0:HD, :, :, h])

        def emit_norm(hc):
            # 1/l rows are in lrows {0,32,64,96}; broadcast across partitions
            # via esel matmul, apply to att
            q0 = 256 * hc
            for c2 in range(2):
                rp = pssm.tile([128, 256], f32, name=f"rp{hc}_{c2}", tag="sm")
                nc.tensor.matmul(rp, lhsT=esel[c2],
                                 rhs=lrows[:, q0:q0 + 256],
                                 start=True, stop=True)
                nc.vector.tensor_tensor(att[c2][:, q0:q0 + 256],
                                        att[c2][:, q0:q0 + 256],
                                        rp, mybir.AluOpType.mult)

        # ---- phase D: output projection per half-chunk (2 query blocks) ----
        y_r = y.rearrange("(o p) n -> p o n", p=128)

        def emit_d(hc, j=None):
            ysb = ypool.tile([128, 2, C], bf16, name=f"ysb{hc}", tag="ysb")
            for j in range(2):
                qt = 2 * hc + j
                for n2 in range(2):
                    yp = pssm.tile([128, 512], f32, name=f"yp{qt}_{n2}", tag="sm")
                    for c2 in range(2):
                        nc.tensor.matmul(yp,
                                         lhsT=att[c2][:, qt * 128:(qt + 1) * 128],
                                         rhs=wo_sb[:, c2, n2 * 512:(n2 + 1) * 512],
                                         start=(c2 == 0), stop=(c2 == 1))
                    if n2 == 0:
                        nc.scalar.copy(ysb[:, j, n2 * 512:(n2 + 1) * 512], yp)
                    else:
                        nc.vector.tensor_copy(ysb[:, j, n2 * 512:(n2 + 1) * 512], yp)
            nc.scalar.dma_start(y_r[:, hc * 2:hc * 2 + 2, :], ysb)

        # ---- emission schedule ----
        # S(kt) leads; PV trails one block so exp+mask land off the PE
        # critical path. Per half-chunk: evac+scatter at kt=2hc+2, broadcast+
        # apply at 2hc+3, projection at 2hc+4 — each stage one iteration
        # behind so the DMA/engine latencies never stall PE. B-span-1 units
        # fill the early iterations.
        def pend(i):
            if i < len(pending):
                pending[i]()

        pend(0)
        emit_b_unit(0, 2, 0)
        pend(1)
        emit_b_unit(0, 0, 0)
        emit_b_vt(0, 0)
        pend(2)
        emit_b_unit(0, 1, 0)
        pend(3)
        fringe = {0: [("b", 0, 2, 1)],
                  1: [("v", 0, 1), ("b", 0, 0, 1)],
                  2: [("b", 0, 1, 1)],
                  4: [("b", 1, 2, 0)],
                  5: [("v", 1, 0), ("b", 1, 0, 0)],
                  6: [("b", 1, 1, 0)],
                  7: [("b", 1, 2, 1), ("d", 0)],
                  8: [("v", 1, 1), ("b", 1, 0, 1)],
                  9: [("b", 1, 1, 1), ("d", 1)],
                  11: [("d", 2)], 13: [("d", 3)], 15: [("d", 4)]}

        # trails (all >= 2 iterations so PE never waits on a fresh non-PE
        # result): PV(kt-2); evac right after PV of an odd block; broadcast+
        # apply (norm) 2 iters after the scatter; projection 2 after norm.
        for kt in range(NKT):
            emit_s(kt)
            if kt == 0:
                pend(4)
            if kt >= 2:
                emit_pv(kt - 2)
                emit_recip(kt - 2)
            for f in fringe.get(kt, ()):
                if f[0] == "b":
                    emit_b_unit(f[1], f[2], f[3])
                elif f[0] == "v":
                    emit_b_vt(f[1], f[2])
                else:
                    emit_d(f[1])
            if kt % 2 == 1 and kt >= 3:
                emit_evac(kt - 2)
            if kt % 2 == 1 and kt >= 5:
                emit_norm((kt - 5) // 2)
        # tail stages, handed to the next rep's early iterations so the
        # serialized normalize/projection chains of the last blocks overlap
        # the next rep's projection phase (the final rep runs them inline)
        return [
            lambda: (emit_pv(NKT - 2), emit_recip(NKT - 2)),
            lambda: (emit_pv(NKT - 1), emit_recip(NKT - 1), emit_evac(NKT - 1)),
            lambda: (emit_norm(6), emit_d(5)),
            lambda: (emit_norm(7), emit_d(6)),
            lambda: emit_d(7),
        ]


def _alibi_slopes(n_heads):
    start = 2.0 ** (-(2.0 ** (-(math.log2(n_heads) - 3))))
    return np.array([start * (start ** i) for i in range(n_heads)], dtype=np.float32)


def kernel(x, Wq, Wk, Wv, Wo):
    global _CACHED_NC
    if _CACHED_NC is None:
        _CACHED_NC = _build_nc()
    nc = _CACHED_NC

    x = np.asarray(x, dtype=np.float32)
    Wq = np.asarray(Wq, dtype=np.float32)
    Wk = np.asarray(Wk, dtype=np.float32)
    Wv = np.asarray(Wv, dtype=np.float32)
    Wo = np.asarray(Wo, dtype=np.float32)

    slopes = _alibi_slopes(H)[:HKV]
    ar = np.arange(T, dtype=np.float32)

    in_maps = []
    for b in range(B):
        xT_b = np.ascontiguousarray(x[b].T.astype(npbf16))
        for g in range(HKV):
            s = float(slopes[g])
            wq_g = Wq[:, g * GH:(g + 1) * GH] * SCALE
            wk_g = Wk[:, g * HD:(g + 1) * HD]
            wv_g = Wv[:, g * HD:(g + 1) * HD]
            wqkv = np.ascontiguousarray(
                np.concatenate([wq_g, wk_g, wv_g], axis=1).astype(npbf16))
            wo_g = np.ascontiguousarray(Wo[g * GH:(g + 1) * GH, :].astype(npbf16))
            negm = -(s * ar + SHIFT)
            aux = np.ascontiguousarray(
                np.stack([np.ones(T, np.float32), np.zeros(T, np.float32),
                          negm, np.ones(T, np.float32),
                          negm, negm, negm, negm]).astype(npbf16))
            sjcol = np.ascontiguousarray((s * ar).reshape(NKT, 128).T)
            in_maps.append({
                "xT": xT_b, "wqkv": wqkv, "wo": wo_g,
                "aux": aux, "sjcol": sjcol,
            })

    global _last_in_maps
    _last_in_maps = in_maps
    res = run_bass_kernel_spmd(nc, in_maps, list(range(B * HKV)))
    out = np.zeros((B, T, C), dtype=np.float32)
    for b in range(B):
        for g in range(HKV):
            out[b] += res.results[b * HKV + g]["y"].astype(np.float32)
    return out
